# revision 1
# baseline (speedup 1.0000x reference)
"""AttentionConv3D Trainium2 kernel.

Computation (per channel c, voxel (d,h,w)):
    q,k,v = 1x1x1 convs of x;  s_kv = q * (k_pad[nbr kv] + rel_bias(c,kv))
    out   = sum_kv softmax_kv(s) * v_pad[nbr kv]         (27 = 3x3x3 window)

Strategy: depth-shard over 8 cores (2 output depth planes each, 1-plane halo).
Host zero-pads x to [64,18,66,66] so the channel-mix matmuls directly produce
zero-padded k/v/q planes. On-device layout: partition p = dl*64 + c
(dl in {0,1} local depth), free dim = padded 66x66 plane (4356).
Per kv-neighbor the window access is a free-dim offset (kh-1)*66 + (kw-1) into
one of three depth-plane buffers K[kd]; the rel bias collapses to a
per-partition scalar B[p, kv], so s = (K_shift + B)*q is ONE DVE
scalar_tensor_tensor op. exp on ACT; numerator/denominator accumulated with
identity matmuls into PSUM on the TensorEngine; 1/den via exp(-ln(den)) on ACT.
"""

import sys
import numpy as np

for _p in ("/opt/trn_rl_repo", "/root/.axon_site/_ro/trn_rl_repo"):
    if _p not in sys.path:
        sys.path.insert(0, _p)

HP = 66               # padded plane edge
HW = HP * HP          # 4356
NPL = 4               # k/v depth planes per core (2 outputs + halo)
R0 = 67               # first interior padded-linear position
CHUNKS = [(67, 1402), (1469, 1536), (3005, 1284)]  # covers [67, 4289); chunk 0's
# window reads ([67-67, 67+67+1402) = [0,1536)) fit inside proj col-chunk 0 so
# the kv loop overlaps the tail of the projection phase.
PROJ = [(0, 1536), (1536, 1536), (3072, 1284)]     # proj psum chunks over 4356
OUT_ROWS = [(0, 21), (21, 44), (44, 64)]           # row bands DMA'd per chunk

# hot-path dtype knobs (fp32 = safe; bf16 halves DVE cost of the e*v path)
E_BF16 = True   # e / v / ev tiles + identity in bf16 (PE still accums fp32)

_CACHE = {}


def _subs(L):
    return [(0, 512), (512, 512), (1024, L - 1024)]


def _build():
    from contextlib import ExitStack
    import concourse.bacc as bacc
    import concourse.tile as tile
    from concourse import mybir

    f32 = mybir.dt.float32
    bf16 = mybir.dt.bfloat16
    edt = bf16 if E_BF16 else f32
    Alu = mybir.AluOpType
    Act = mybir.ActivationFunctionType

    nc = bacc.Bacc("TRN2", target_bir_lowering=False)
    xs_d = nc.dram_tensor("xs", [64, NPL * HW], f32, kind="ExternalInput")
    wk_d = nc.dram_tensor("wk2", [64, 128], f32, kind="ExternalInput")
    wv_d = nc.dram_tensor("wv2", [64, 128], f32, kind="ExternalInput")
    wq_d = nc.dram_tensor("wq2", [64, 128], f32, kind="ExternalInput")
    b_d = nc.dram_tensor("bias", [128, 27], f32, kind="ExternalInput")
    id_d = nc.dram_tensor("ident", [128, 128], edt, kind="ExternalInput")
    out_d = nc.dram_tensor("out", [128, 64, 64], f32, kind="ExternalOutput")

    with tile.TileContext(nc) as tc, ExitStack() as ctx:
        singles = ctx.enter_context(tc.tile_pool(name="singles", bufs=1))
        planes = ctx.enter_context(tc.tile_pool(name="planes", bufs=1))
        wpool = ctx.enter_context(tc.tile_pool(name="work", bufs=2))

        wk_s = singles.tile([64, 128], f32, tag="wk")
        wv_s = singles.tile([64, 128], f32, tag="wv")
        wq_s = singles.tile([64, 128], f32, tag="wq")
        id_s = singles.tile([128, 128], edt, tag="id")
        b_s = singles.tile([128, 27], f32, tag="b")
        ebias = singles.tile([128, 1], f32, tag="ebias")
        nc.vector.memset(ebias[:], -28.0)
        for t, d in ((wk_s, wk_d), (wv_s, wv_d), (wq_s, wq_d),
                     (id_s, id_d), (b_s, b_d)):
            nc.sync.dma_start(t[:], d[:])

        Kp = [planes.tile([128, HW], f32, tag=f"k{i}", name=f"k{i}") for i in range(3)]
        Vp = [planes.tile([128, HW], edt, tag=f"v{i}", name=f"v{i}") for i in range(3)]
        Q = planes.tile([128, HW], f32, tag="q")
        OUT = planes.tile([128, HW], f32, tag="o")

        # ---- projections: plane m of xs -> k/v (dual-copy weights give the
        # same output plane on partitions 0:64 and 64:128), q for m in {1,2}.
        # column-chunk OUTER so all planes' first 1536 columns (what kv chunk 0
        # needs) are projected before any plane's later columns.
        with tc.tile_pool(name="xp", bufs=3) as xpool, \
             tc.tile_pool(name="pp", bufs=2, space="PSUM") as ppool:
            for base, L3 in PROJ:
                for m in range(NPL):
                    X = xpool.tile([64, 1536], f32, tag="x")
                    nc.sync.dma_start(X[:, :L3],
                                      xs_d[:, m * HW + base:m * HW + base + L3])
                    projs = [(wk_s, "k"), (wv_s, "v")]
                    if m in (1, 2):
                        projs.append((wq_s, "q"))
                    for w_s, kind in projs:
                        pp = ppool.tile([128, 1536], f32, tag="pp")
                        for a, bl in _subs(L3):
                            nc.tensor.matmul(pp[:, a:a + bl], w_s[:],
                                             X[:, a:a + bl],
                                             start=True, stop=True)
                        sl = (slice(0, 64), slice(base, base + L3))
                        sh = (slice(64, 128), slice(base, base + L3))
                        if kind == "k":
                            # split k evacuations across DVE/ACT to keep DVE,
                            # the span-limiting engine, under ACT's load
                            if m <= 2:
                                nc.vector.tensor_copy(Kp[m][sl], pp[0:64, :L3])
                            if m >= 1:
                                nc.scalar.copy(Kp[m - 1][sh], pp[64:128, :L3])
                        elif kind == "v":
                            if m <= 2:
                                nc.scalar.copy(Vp[m][sl], pp[0:64, :L3])
                            if m >= 1:
                                nc.scalar.copy(Vp[m - 1][sh], pp[64:128, :L3])
                        elif m == 1:
                            nc.vector.tensor_copy(Q[sl], pp[0:64, :L3])
                        else:
                            nc.scalar.copy(Q[sh], pp[64:128, :L3])

        # ---- 27-neighbor softmax attention, PSUM-chunked over the plane
        accp = ctx.enter_context(tc.tile_pool(name="acc", bufs=1, space="PSUM"))
        OUTv = OUT.rearrange("p (r c) -> p r c", c=HP)
        GPSET = frozenset((0, 2, 6, 8, 9, 11, 15, 17, 18, 20, 21, 23, 24, 26))
        for (c0, L), (r0, r1) in zip(CHUNKS, OUT_ROWS):
            den = accp.tile([128, 1536], f32, tag="den")
            num = accp.tile([128, 1536], f32, tag="num")
            for kv in range(27):
                kd, r = divmod(kv, 9)
                kh, kw = divmod(r, 3)
                dd = (kh - 1) * HP + (kw - 1)
                s_t = wpool.tile([128, 1536], f32, tag="s")
                nc.vector.scalar_tensor_tensor(
                    s_t[:, :L], Kp[kd][:, c0 + dd:c0 + dd + L],
                    b_s[:, kv:kv + 1], Q[:, c0:c0 + L], Alu.add, Alu.mult)
                e_t = wpool.tile([128, 1536], edt, tag="e")
                # bias keeps exp inside the ACT table range (softmax is
                # shift-invariant; the -28 cancels via the ln/exp normalize)
                nc.scalar.activation(e_t[:, :L], s_t[:, :L], Act.Exp, bias=ebias[:])
                ev_t = wpool.tile([128, 1536], edt, tag="ev")
                # DVE is the bottleneck engine; hand ~half the e*v products
                # to the otherwise-idle GPSIMD (stock Q7 tensor_tensor).
                ev_eng = nc.gpsimd if (kw == 1 or kv in GPSET) else nc.vector
                ev_eng.tensor_mul(ev_t[:, :L], e_t[:, :L],
                                  Vp[kd][:, c0 + dd:c0 + dd + L])
                st, sp = kv == 0, kv == 26
                for a, bl in _subs(L):
                    nc.tensor.matmul(den[:, a:a + bl], id_s[:], e_t[:, a:a + bl],
                                     start=st, stop=sp)
                    nc.tensor.matmul(num[:, a:a + bl], id_s[:], ev_t[:, a:a + bl],
                                     start=st, stop=sp)
            l_t = wpool.tile([128, 1536], f32, tag="s")
            nc.scalar.activation(l_t[:, :L], den[:, :L], Act.Ln)
            f_t = wpool.tile([128, 1536], f32, tag="f")
            nc.scalar.activation(f_t[:, :L], l_t[:, :L], Act.Exp, scale=-1.0)
            nc.vector.tensor_mul(OUT[:, c0:c0 + L], num[:, :L], f_t[:, :L])
            # rows fully covered by chunks <= this one stream out immediately
            nc.sync.dma_start(out_d[:, r0:r1, :],
                              OUTv[:, 1 + r0:1 + r1, 1:65])
    nc.finalize()
    return nc


def kernel(x, w_q, w_k, w_v, rel_d, rel_h, rel_w):
    from concourse.bass_utils import run_bass_kernel_spmd

    x = np.asarray(x, np.float32)
    rd = np.asarray(rel_d, np.float32).reshape(21, 3)
    rh = np.asarray(rel_h, np.float32).reshape(21, 3)
    rw = np.asarray(rel_w, np.float32).reshape(22, 3)

    xp = np.zeros((64, 18, HP, HP), np.float32)
    xp[:, 1:17, 1:65, 1:65] = x[0]

    B = np.zeros((128, 27), np.float32)
    for c in range(64):
        for kv in range(27):
            kd, r = divmod(kv, 9)
            kh, kw = divmod(r, 3)
            b = rd[c, kd] if c < 21 else (rh[c - 21, kh] if c < 42 else rw[c - 42, kw])
            B[c, kv] = B[64 + c, kv] = b

    idt = np.eye(128, dtype=np.float32)
    if E_BF16:
        import ml_dtypes
        idt = idt.astype(ml_dtypes.bfloat16)
    com = {
        "wk2": np.concatenate([w_k.T, w_k.T], 1).astype(np.float32).copy(),
        "wv2": np.concatenate([w_v.T, w_v.T], 1).astype(np.float32).copy(),
        "wq2": np.concatenate([w_q.T, w_q.T], 1).astype(np.float32).copy(),
        "bias": B, "ident": idt,
    }
    in_maps = []
    for i in range(8):
        m = dict(com)
        m["xs"] = xp[:, 2 * i:2 * i + 4].reshape(64, NPL * HW).copy()
        in_maps.append(m)

    if "nc" not in _CACHE:
        _CACHE["nc"] = _build()
    res = run_bass_kernel_spmd(_CACHE["nc"], in_maps, list(range(8)))

    out = np.empty((1, 64, 16, 64, 64), np.float32)
    for i in range(8):
        arr = res.results[i]["out"].reshape(2, 64, 64, 64)
        out[0, :, 2 * i] = arr[0]
        out[0, :, 2 * i + 1] = arr[1]
    return out



# revision 2
# speedup vs baseline: 2.0616x; 2.0616x over previous
"""AttentionConv3D Trainium2 kernel.

Computation (per channel c, voxel (d,h,w)):
    q,k,v = 1x1x1 convs of x;  s_kv = q * (k_pad[nbr kv] + rel_bias(c,kv))
    out   = sum_kv softmax_kv(s) * v_pad[nbr kv]         (27 = 3x3x3 window)

Strategy: depth-shard over 8 cores (2 output depth planes each, 1-plane halo).
Host zero-pads x to [64,18,66,66] so the channel-mix matmuls directly produce
zero-padded k/v/q planes. On-device layout: partition p = dl*64 + c
(dl in {0,1} local depth), free dim = padded 66x66 plane (4356).
Per kv-neighbor the window access is a free-dim offset (kh-1)*66 + (kw-1) into
one of three depth-plane buffers K[kd]; the rel bias collapses to a
per-partition scalar B[p, kv], so s = (K_shift + B)*q is ONE DVE
scalar_tensor_tensor op. exp on ACT; numerator/denominator accumulated with
identity matmuls into PSUM on the TensorEngine; 1/den via exp(-ln(den)) on ACT.

The wall clock is dominated by the ~38MB/s axon tunnel, so I/O is minimized:
x is uploaded fp16 (projection matmuls run fp16xfp16 -> fp32 PSUM), the output
is downloaded fp16 and upconverted on host, the donated output buffers are
device-resident (recycled between calls, never re-uploaded), and the jitted
dispatch closure is built once and cached.
"""

import sys
import numpy as np

for _p in ("/opt/trn_rl_repo", "/root/.axon_site/_ro/trn_rl_repo"):
    if _p not in sys.path:
        sys.path.insert(0, _p)

HP = 66               # padded plane edge
HW = HP * HP          # 4356
NPL = 4               # k/v depth planes per core (2 outputs + halo)
R0 = 67               # first interior padded-linear position
CHUNKS = [(67, 1402), (1469, 1536), (3005, 1284)]  # covers [67, 4289); chunk 0's
# window reads ([67-67, 67+67+1402) = [0,1536)) fit inside proj col-chunk 0 so
# the kv loop overlaps the tail of the projection phase.
PROJ = [(0, 1536), (1536, 1536), (3072, 1284)]     # proj psum chunks over 4356
OUT_ROWS = [(0, 21), (21, 44), (44, 64)]           # row bands DMA'd per chunk

# hot-path dtype knobs (fp32 = safe; bf16 halves DVE cost of the e*v path)
E_BF16 = True   # e / v / ev tiles + identity in bf16 (PE still accums fp32)

N_CORES = 8

_CACHE = {}


def _subs(L):
    return [(0, 512), (512, 512), (1024, L - 1024)]


def _build():
    from contextlib import ExitStack
    import concourse.bacc as bacc
    import concourse.tile as tile
    from concourse import mybir

    f32 = mybir.dt.float32
    f16 = mybir.dt.float16
    bf16 = mybir.dt.bfloat16
    edt = bf16 if E_BF16 else f32
    Alu = mybir.AluOpType
    Act = mybir.ActivationFunctionType

    nc = bacc.Bacc("TRN2", target_bir_lowering=False)
    xs_d = nc.dram_tensor("xs", [64, NPL * HW], f16, kind="ExternalInput")
    wk_d = nc.dram_tensor("wk2", [64, 128], f16, kind="ExternalInput")
    wv_d = nc.dram_tensor("wv2", [64, 128], f16, kind="ExternalInput")
    wq_d = nc.dram_tensor("wq2", [64, 128], f16, kind="ExternalInput")
    b_d = nc.dram_tensor("bias", [128, 27], f32, kind="ExternalInput")
    id_d = nc.dram_tensor("ident", [128, 128], edt, kind="ExternalInput")
    out_d = nc.dram_tensor("out", [128, 64, 64], f16, kind="ExternalOutput")

    with tile.TileContext(nc) as tc, ExitStack() as ctx:
        singles = ctx.enter_context(tc.tile_pool(name="singles", bufs=1))
        planes = ctx.enter_context(tc.tile_pool(name="planes", bufs=1))
        wpool = ctx.enter_context(tc.tile_pool(name="work", bufs=2))

        wk_s = singles.tile([64, 128], f16, tag="wk")
        wv_s = singles.tile([64, 128], f16, tag="wv")
        wq_s = singles.tile([64, 128], f16, tag="wq")
        id_s = singles.tile([128, 128], edt, tag="id")
        b_s = singles.tile([128, 27], f32, tag="b")
        ebias = singles.tile([128, 1], f32, tag="ebias")
        nc.vector.memset(ebias[:], -28.0)
        for t, d in ((wk_s, wk_d), (wv_s, wv_d), (wq_s, wq_d),
                     (id_s, id_d), (b_s, b_d)):
            nc.sync.dma_start(t[:], d[:])

        Kp = [planes.tile([128, HW], f32, tag=f"k{i}", name=f"k{i}") for i in range(3)]
        Vp = [planes.tile([128, HW], edt, tag=f"v{i}", name=f"v{i}") for i in range(3)]
        Q = planes.tile([128, HW], f32, tag="q")
        OUT = planes.tile([128, HW], f16, tag="o")

        # ---- projections: plane m of xs -> k/v (dual-copy weights give the
        # same output plane on partitions 0:64 and 64:128), q for m in {1,2}.
        # column-chunk OUTER so all planes' first 1536 columns (what kv chunk 0
        # needs) are projected before any plane's later columns.
        with tc.tile_pool(name="xp", bufs=3) as xpool, \
             tc.tile_pool(name="pp", bufs=2, space="PSUM") as ppool:
            for base, L3 in PROJ:
                for m in range(NPL):
                    X = xpool.tile([64, 1536], f16, tag="x")
                    nc.sync.dma_start(X[:, :L3],
                                      xs_d[:, m * HW + base:m * HW + base + L3])
                    projs = [(wk_s, "k"), (wv_s, "v")]
                    if m in (1, 2):
                        projs.append((wq_s, "q"))
                    for w_s, kind in projs:
                        pp = ppool.tile([128, 1536], f32, tag="pp")
                        for a, bl in _subs(L3):
                            nc.tensor.matmul(pp[:, a:a + bl], w_s[:],
                                             X[:, a:a + bl],
                                             start=True, stop=True)
                        sl = (slice(0, 64), slice(base, base + L3))
                        sh = (slice(64, 128), slice(base, base + L3))
                        if kind == "k":
                            # split k evacuations across DVE/ACT to keep DVE,
                            # the span-limiting engine, under ACT's load
                            if m <= 2:
                                nc.vector.tensor_copy(Kp[m][sl], pp[0:64, :L3])
                            if m >= 1:
                                nc.scalar.copy(Kp[m - 1][sh], pp[64:128, :L3])
                        elif kind == "v":
                            if m <= 2:
                                nc.scalar.copy(Vp[m][sl], pp[0:64, :L3])
                            if m >= 1:
                                nc.scalar.copy(Vp[m - 1][sh], pp[64:128, :L3])
                        elif m == 1:
                            nc.vector.tensor_copy(Q[sl], pp[0:64, :L3])
                        else:
                            nc.scalar.copy(Q[sh], pp[64:128, :L3])

        # ---- 27-neighbor softmax attention, PSUM-chunked over the plane
        accp = ctx.enter_context(tc.tile_pool(name="acc", bufs=1, space="PSUM"))
        OUTv = OUT.rearrange("p (r c) -> p r c", c=HP)
        GPSET = frozenset((0, 2, 6, 8, 9, 11, 15, 17, 18, 20, 21, 23, 24, 26))
        for (c0, L), (r0, r1) in zip(CHUNKS, OUT_ROWS):
            den = accp.tile([128, 1536], f32, tag="den")
            num = accp.tile([128, 1536], f32, tag="num")
            for kv in range(27):
                kd, r = divmod(kv, 9)
                kh, kw = divmod(r, 3)
                dd = (kh - 1) * HP + (kw - 1)
                s_t = wpool.tile([128, 1536], f32, tag="s")
                nc.vector.scalar_tensor_tensor(
                    s_t[:, :L], Kp[kd][:, c0 + dd:c0 + dd + L],
                    b_s[:, kv:kv + 1], Q[:, c0:c0 + L], Alu.add, Alu.mult)
                e_t = wpool.tile([128, 1536], edt, tag="e")
                # bias keeps exp inside the ACT table range (softmax is
                # shift-invariant; the -28 cancels via the ln/exp normalize)
                nc.scalar.activation(e_t[:, :L], s_t[:, :L], Act.Exp, bias=ebias[:])
                ev_t = wpool.tile([128, 1536], edt, tag="ev")
                # DVE is the bottleneck engine; hand ~half the e*v products
                # to the otherwise-idle GPSIMD (stock Q7 tensor_tensor).
                ev_eng = nc.gpsimd if (kw == 1 or kv in GPSET) else nc.vector
                ev_eng.tensor_mul(ev_t[:, :L], e_t[:, :L],
                                  Vp[kd][:, c0 + dd:c0 + dd + L])
                st, sp = kv == 0, kv == 26
                for a, bl in _subs(L):
                    nc.tensor.matmul(den[:, a:a + bl], id_s[:], e_t[:, a:a + bl],
                                     start=st, stop=sp)
                    nc.tensor.matmul(num[:, a:a + bl], id_s[:], ev_t[:, a:a + bl],
                                     start=st, stop=sp)
            l_t = wpool.tile([128, 1536], f32, tag="s")
            nc.scalar.activation(l_t[:, :L], den[:, :L], Act.Ln)
            f_t = wpool.tile([128, 1536], f32, tag="f")
            nc.scalar.activation(f_t[:, :L], l_t[:, :L], Act.Exp, scale=-1.0)
            nc.vector.tensor_mul(OUT[:, c0:c0 + L], num[:, :L], f_t[:, :L])
            # rows fully covered by chunks <= this one stream out immediately
            nc.sync.dma_start(out_d[:, r0:r1, :],
                              OUTv[:, 1 + r0:1 + r1, 1:65])
    nc.finalize()
    return nc


def _compile():
    """Build the Bass module once and cache a persistent jitted dispatcher.

    run_bass_kernel_spmd re-creates (and re-traces) its jit closure on every
    call; building it once here removes that per-call overhead and lets us
    keep the donated output buffers device-resident between calls.
    """
    import jax
    from concourse import mybir
    from concourse.bass2jax import (_bass_exec_p, partition_id_tensor,
                                    install_neuronx_cc_hook)
    from jax.sharding import Mesh, PartitionSpec
    from jax.experimental.shard_map import shard_map

    install_neuronx_cc_hook()
    nc = _build()

    partition_name = nc.partition_id_tensor.name if nc.partition_id_tensor else None
    in_names, out_names, out_avals, zero_outs = [], [], [], []
    for alloc in nc.m.functions[0].allocations:
        if not isinstance(alloc, mybir.MemoryLocationSet):
            continue
        name = alloc.memorylocations[0].name
        if alloc.kind == "ExternalInput":
            if name != partition_name:
                in_names.append(name)
        elif alloc.kind == "ExternalOutput":
            shape = tuple(alloc.tensor_shape)
            dtype = mybir.dt.np(alloc.dtype)
            out_avals.append(jax.core.ShapedArray(shape, dtype))
            out_names.append(name)
            zero_outs.append(np.zeros((N_CORES * shape[0], *shape[1:]), dtype))
    n_params = len(in_names)
    n_outs = len(out_avals)
    in_names_full = list(in_names) + out_names
    if partition_name is not None:
        in_names_full.append(partition_name)
    donate = tuple(range(n_params, n_params + n_outs))

    def _body(*args):
        operands = list(args)
        if partition_name is not None:
            operands.append(partition_id_tensor())
        outs = _bass_exec_p.bind(
            *operands,
            out_avals=tuple(out_avals),
            in_names=tuple(in_names_full),
            out_names=tuple(out_names),
            lowering_input_output_aliases=(),
            sim_require_finite=True,
            sim_require_nnan=True,
            nc=nc,
        )
        return tuple(outs)

    devices = jax.devices()[:N_CORES]
    mesh = Mesh(np.asarray(devices), ("core",))
    in_specs = (PartitionSpec("core"),) * (n_params + n_outs)
    out_specs = (PartitionSpec("core"),) * n_outs
    fn = jax.jit(
        shard_map(_body, mesh=mesh, in_specs=in_specs, out_specs=out_specs,
                  check_rep=False),
        donate_argnums=donate,
        keep_unused=True,
    )
    _CACHE.update(nc=nc, fn=fn, in_names=in_names, prev_outs=zero_outs,
                  n_outs=n_outs)


def _host_inputs(x, w_q, w_k, w_v, rel_d, rel_h, rel_w):
    """Per-core inputs, already concatenated along axis 0 (shard layout)."""
    rd = np.asarray(rel_d, np.float32).reshape(21, 3)
    rh = np.asarray(rel_h, np.float32).reshape(21, 3)
    rw = np.asarray(rel_w, np.float32).reshape(22, 3)

    xpad = np.zeros((64, 18, HP, HP), np.float16)
    xpad[:, 1:17, 1:65, 1:65] = x[0]
    xs_g = np.empty((N_CORES, 64, NPL * HW), np.float16)
    for i in range(N_CORES):
        xs_g[i] = xpad[:, 2 * i:2 * i + 4].reshape(64, NPL * HW)

    kv27 = np.arange(27)
    kd_i, kh_i, kw_i = kv27 // 9, (kv27 // 3) % 3, kv27 % 3
    B64 = np.empty((64, 27), np.float32)
    B64[:21] = rd[:, kd_i]
    B64[21:42] = rh[:, kh_i]
    B64[42:] = rw[:, kw_i]
    B = np.concatenate([B64, B64], 0)

    import ml_dtypes
    idt = np.eye(128, dtype=np.float32)
    idt = idt.astype(ml_dtypes.bfloat16 if E_BF16 else np.float32)

    def dup(w):
        w2 = np.concatenate([w.T, w.T], 1).astype(np.float16)
        return np.tile(w2, (N_CORES, 1))

    return {
        "xs": xs_g.reshape(N_CORES * 64, NPL * HW),
        "wk2": dup(np.asarray(w_k)),
        "wv2": dup(np.asarray(w_v)),
        "wq2": dup(np.asarray(w_q)),
        "bias": np.tile(B, (N_CORES, 1)),
        "ident": np.tile(idt, (N_CORES, 1)),
    }


def kernel(x, w_q, w_k, w_v, rel_d, rel_h, rel_w):
    import jax

    if "fn" not in _CACHE:
        _compile()

    gmaps = _host_inputs(np.asarray(x, np.float32), w_q, w_k, w_v,
                         rel_d, rel_h, rel_w)
    args = [gmaps[nm] for nm in _CACHE["in_names"]]
    out_arrs = _CACHE["fn"](*args, *_CACHE["prev_outs"])
    # recycle the device-resident output buffers as next call's donation args
    # (their contents are irrelevant: the NEFF writes every output element)
    _CACHE["prev_outs"] = list(out_arrs)

    o = np.asarray(out_arrs[0])          # [8*128, 64, 64] fp16
    og = o.reshape(N_CORES, 2, 64, 64, 64)
    out = og.transpose(2, 0, 1, 3, 4).reshape(1, 64, 16, 64, 64)
    return out.astype(np.float32)


# revision 5
# speedup vs baseline: 2.3541x; 1.1419x over previous
"""AttentionConv3D Trainium2 kernel.

Computation (per channel c, voxel (d,h,w)):
    q,k,v = 1x1x1 convs of x;  s_kv = q * (k_pad[nbr kv] + rel_bias(c,kv))
    out   = sum_kv softmax_kv(s) * v_pad[nbr kv]         (27 = 3x3x3 window)

Strategy: depth-shard over 8 cores (2 output depth planes each, 1-plane halo).
On-device layout: partition p = dl*64 + c (dl in {0,1} local depth), free dim
= zero-padded 66x66 plane (4356). Per kv-neighbor the window access is a
free-dim offset (kh-1)*66 + (kw-1) into one of three depth-plane buffers
K[kd]; the rel bias collapses to a per-partition scalar B[p, kv], so
s = (K_shift + B)*q is ONE DVE scalar_tensor_tensor op. exp on ACT;
numerator/denominator accumulated with identity matmuls into PSUM on the
TensorEngine; 1/den via exp(-ln(den)) on ACT.

The wall clock is dominated by the ~40MB/s (half-duplex) axon tunnel, so I/O
is minimized:
 - each core uploads ONLY its two fp16 depth planes (no halo duplication);
   the 1-plane halos are exchanged on device: a world AllGather of every
   core's plane pair, then a per-core one-hot masked sum (host-uploaded
   selection scalars, 16 DVE select-accumulate ops) picks the two neighbor
   planes — edge cores get all-zero masks, i.e. free zero padding.
 - projection matmuls run fp16 x fp16 -> fp32 PSUM.
 - the output is downloaded fp16 and upconverted on host.
 - donated output buffers are device-resident (recycled between calls).
 - the jitted dispatch closure is built once and cached.
"""

import sys
import numpy as np

for _p in ("/opt/trn_rl_repo", "/root/.axon_site/_ro/trn_rl_repo"):
    if _p not in sys.path:
        sys.path.insert(0, _p)

HP = 66               # padded plane edge
HW = HP * HP          # 4356
CHUNKS = [(67, 1402), (1469, 1536), (3005, 1284)]  # covers [67, 4289); chunk 0's
# window reads ([67-67, 67+67+1402) = [0,1536)) fit inside proj col-chunk 0 so
# the kv loop overlaps the tail of the projection phase.
PROJ = [(0, 1536), (1536, 1536), (3072, 1284)]     # proj psum chunks over 4356
OUT_ROWS = [(0, 21), (21, 44), (44, 64)]           # row bands DMA'd per chunk
ZW = 1089             # halo-assembly column chunk (4 chunks cover 4356)

# hot-path dtype knobs (fp32 = safe; bf16 halves DVE cost of the e*v path)
E_BF16 = True   # e / v / ev tiles + identity in bf16 (PE still accums fp32)

N_CORES = 8

_CACHE = {}


def _subs(L):
    return [(0, 512), (512, 512), (1024, L - 1024)]


def _build():
    from contextlib import ExitStack
    import concourse.bacc as bacc
    import concourse.tile as tile
    from concourse import mybir

    f32 = mybir.dt.float32
    f16 = mybir.dt.float16
    bf16 = mybir.dt.bfloat16
    edt = bf16 if E_BF16 else f32
    Alu = mybir.AluOpType
    Act = mybir.ActivationFunctionType

    nc = bacc.Bacc("TRN2", target_bir_lowering=False)
    # own two padded depth planes, partition = slot*64 + channel
    bnd_d = nc.dram_tensor("bnd", [128, HW], f16, kind="ExternalInput")
    # halo selection scalars: rows 0:64 pick the left-halo plane, 64:128 the
    # right-halo plane, as one-hot over the 16 gathered planes
    hs_d = nc.dram_tensor("hsel", [128, 16], f32, kind="ExternalInput")
    wk_d = nc.dram_tensor("wk2", [64, 128], f16, kind="ExternalInput")
    wv_d = nc.dram_tensor("wv2", [64, 128], f16, kind="ExternalInput")
    wq_d = nc.dram_tensor("wq2", [64, 128], f16, kind="ExternalInput")
    b_d = nc.dram_tensor("bias", [128, 27], f32, kind="ExternalInput")
    id_d = nc.dram_tensor("ident", [128, 128], edt, kind="ExternalInput")
    out_d = nc.dram_tensor("out", [128, 64, 64], f16, kind="ExternalOutput")

    # collective staging (collectives can't touch I/O tensors directly)
    bb_d = nc.dram_tensor("bb", [128, HW], f16)
    g_d = nc.dram_tensor("g", [16, 64, HW], f16, addr_space="Shared")

    with tile.TileContext(nc) as tc, ExitStack() as ctx:
        singles = ctx.enter_context(tc.tile_pool(name="singles", bufs=1))
        planes = ctx.enter_context(tc.tile_pool(name="planes", bufs=1))
        wpool = ctx.enter_context(tc.tile_pool(name="work", bufs=2))

        wk_s = singles.tile([64, 128], f16, tag="wk")
        wv_s = singles.tile([64, 128], f16, tag="wv")
        wq_s = singles.tile([64, 128], f16, tag="wq")
        id_s = singles.tile([128, 128], edt, tag="id")
        b_s = singles.tile([128, 27], f32, tag="b")
        hs_s = singles.tile([128, 16], f32, tag="hs")
        ebias = singles.tile([128, 1], f32, tag="ebias")
        nc.vector.memset(ebias[:], -28.0)
        for t, d in ((wk_s, wk_d), (wv_s, wv_d), (wq_s, wq_d),
                     (id_s, id_d), (b_s, b_d), (hs_s, hs_d)):
            nc.sync.dma_start(t[:], d[:])

        # ---- halo exchange: world AllGather of everyone's plane pair
        nc.gpsimd.dma_start(bb_d[:], bnd_d[:])
        nc.gpsimd.collective_compute(
            "AllGather", mybir.AluOpType.bypass,
            replica_groups=[list(range(N_CORES))],
            ins=[bb_d[:]], outs=[g_d[:]])

        # own planes straight from the input; halo planes assembled below
        X1 = planes.tile([64, HW], f16, tag="x1")
        X2 = planes.tile([64, HW], f16, tag="x2")
        XH = planes.tile([128, HW], f16, tag="xh")   # 0:64 = X0, 64:128 = X3
        nc.sync.dma_start(X1[:], bnd_d[0:64, :])
        nc.sync.dma_start(X2[:], bnd_d[64:128, :])

        with tc.tile_pool(name="gt", bufs=1) as gpool:
            for w in range(4):
                ws = w * ZW
                GT = gpool.tile([128, 16 * ZW], f16, tag="gt")
                GTv = GT.rearrange("p (j z) -> p j z", j=16)
                src = g_d[:, :, ws:ws + ZW].transpose([1, 0, 2])
                nc.sync.dma_start(GTv[0:64, :, :], src)
                nc.sync.dma_start(GTv[64:128, :, :], src)
                # one-hot select-accumulate over the 16 gathered planes
                nc.vector.tensor_scalar_mul(
                    XH[:, ws:ws + ZW], GT[:, 0:ZW], hs_s[:, 0:1])
                for j in range(1, 16):
                    nc.vector.scalar_tensor_tensor(
                        XH[:, ws:ws + ZW], GT[:, j * ZW:(j + 1) * ZW],
                        hs_s[:, j:j + 1], XH[:, ws:ws + ZW],
                        Alu.mult, Alu.add)

        # matmul needs moving operands at base partition 0: relocate the
        # right-halo half of XH down via one SBUF->SBUF DMA
        X3 = planes.tile([64, HW], f16, tag="x3")
        nc.sync.dma_start(X3[:], XH[64:128, :])

        Kp = [planes.tile([128, HW], f32, tag=f"k{i}", name=f"k{i}") for i in range(3)]
        Vp = [planes.tile([128, HW], edt, tag=f"v{i}", name=f"v{i}") for i in range(3)]
        Q = planes.tile([128, HW], f32, tag="q")
        OUT = planes.tile([128, HW], f16, tag="o")

        # ---- projections: plane m -> k/v (dual-copy weights give the same
        # output plane on partitions 0:64 and 64:128), q for m in {1,2}.
        # column-chunk OUTER so all planes' first 1536 columns (what kv chunk 0
        # needs) are projected before any plane's later columns.
        Xsrc = [XH[0:64, :], X1[:], X2[:], X3[:]]
        with tc.tile_pool(name="pp", bufs=2, space="PSUM") as ppool:
            for base, L3 in PROJ:
                for m in range(4):
                    X = Xsrc[m]
                    projs = [(wk_s, "k"), (wv_s, "v")]
                    if m in (1, 2):
                        projs.append((wq_s, "q"))
                    for w_s, kind in projs:
                        pp = ppool.tile([128, 1536], f32, tag="pp")
                        for a, bl in _subs(L3):
                            nc.tensor.matmul(pp[:, a:a + bl], w_s[:],
                                             X[:, base + a:base + a + bl],
                                             start=True, stop=True)
                        sl = (slice(0, 64), slice(base, base + L3))
                        sh = (slice(64, 128), slice(base, base + L3))
                        if kind == "k":
                            # split k evacuations across DVE/ACT to keep DVE,
                            # the span-limiting engine, under ACT's load
                            if m <= 2:
                                nc.vector.tensor_copy(Kp[m][sl], pp[0:64, :L3])
                            if m >= 1:
                                nc.scalar.copy(Kp[m - 1][sh], pp[64:128, :L3])
                        elif kind == "v":
                            if m <= 2:
                                nc.scalar.copy(Vp[m][sl], pp[0:64, :L3])
                            if m >= 1:
                                nc.scalar.copy(Vp[m - 1][sh], pp[64:128, :L3])
                        elif m == 1:
                            nc.vector.tensor_copy(Q[sl], pp[0:64, :L3])
                        else:
                            nc.scalar.copy(Q[sh], pp[64:128, :L3])

        # ---- 27-neighbor softmax attention, PSUM-chunked over the plane
        accp = ctx.enter_context(tc.tile_pool(name="acc", bufs=1, space="PSUM"))
        OUTv = OUT.rearrange("p (r c) -> p r c", c=HP)
        GPSET = frozenset((0, 2, 6, 8, 9, 11, 15, 17, 18, 20, 21, 23, 24, 26))
        for (c0, L), (r0, r1) in zip(CHUNKS, OUT_ROWS):
            den = accp.tile([128, 1536], f32, tag="den")
            num = accp.tile([128, 1536], f32, tag="num")
            for kv in range(27):
                kd, r = divmod(kv, 9)
                kh, kw = divmod(r, 3)
                dd = (kh - 1) * HP + (kw - 1)
                s_t = wpool.tile([128, 1536], f32, tag="s")
                nc.vector.scalar_tensor_tensor(
                    s_t[:, :L], Kp[kd][:, c0 + dd:c0 + dd + L],
                    b_s[:, kv:kv + 1], Q[:, c0:c0 + L], Alu.add, Alu.mult)
                e_t = wpool.tile([128, 1536], edt, tag="e")
                # bias keeps exp inside the ACT table range (softmax is
                # shift-invariant; the -28 cancels via the ln/exp normalize)
                nc.scalar.activation(e_t[:, :L], s_t[:, :L], Act.Exp, bias=ebias[:])
                ev_t = wpool.tile([128, 1536], edt, tag="ev")
                # DVE is the bottleneck engine; hand ~half the e*v products
                # to the otherwise-idle GPSIMD (stock Q7 tensor_tensor).
                ev_eng = nc.gpsimd if (kw == 1 or kv in GPSET) else nc.vector
                ev_eng.tensor_mul(ev_t[:, :L], e_t[:, :L],
                                  Vp[kd][:, c0 + dd:c0 + dd + L])
                st, sp = kv == 0, kv == 26
                for a, bl in _subs(L):
                    nc.tensor.matmul(den[:, a:a + bl], id_s[:], e_t[:, a:a + bl],
                                     start=st, stop=sp)
                    nc.tensor.matmul(num[:, a:a + bl], id_s[:], ev_t[:, a:a + bl],
                                     start=st, stop=sp)
            l_t = wpool.tile([128, 1536], f32, tag="s")
            nc.scalar.activation(l_t[:, :L], den[:, :L], Act.Ln)
            f_t = wpool.tile([128, 1536], f32, tag="f")
            nc.scalar.activation(f_t[:, :L], l_t[:, :L], Act.Exp, scale=-1.0)
            nc.vector.tensor_mul(OUT[:, c0:c0 + L], num[:, :L], f_t[:, :L])
            # rows fully covered by chunks <= this one stream out immediately
            nc.sync.dma_start(out_d[:, r0:r1, :],
                              OUTv[:, 1 + r0:1 + r1, 1:65])
    nc.finalize()
    return nc


def _compile():
    """Build the Bass module once and cache a persistent jitted dispatcher.

    run_bass_kernel_spmd re-creates (and re-traces) its jit closure on every
    call; building it once here removes that per-call overhead and lets us
    keep the donated output buffers device-resident between calls.
    """
    import jax
    from concourse import mybir
    from concourse.bass2jax import (_bass_exec_p, partition_id_tensor,
                                    install_neuronx_cc_hook)
    from jax.sharding import Mesh, PartitionSpec
    from jax.experimental.shard_map import shard_map

    install_neuronx_cc_hook()
    nc = _build()

    partition_name = nc.partition_id_tensor.name if nc.partition_id_tensor else None
    in_names, out_names, out_avals, zero_outs = [], [], [], []
    for alloc in nc.m.functions[0].allocations:
        if not isinstance(alloc, mybir.MemoryLocationSet):
            continue
        name = alloc.memorylocations[0].name
        if alloc.kind == "ExternalInput":
            if name != partition_name:
                in_names.append(name)
        elif alloc.kind == "ExternalOutput":
            shape = tuple(alloc.tensor_shape)
            dtype = mybir.dt.np(alloc.dtype)
            out_avals.append(jax.core.ShapedArray(shape, dtype))
            out_names.append(name)
            zero_outs.append(np.zeros((N_CORES * shape[0], *shape[1:]), dtype))
    n_params = len(in_names)
    n_outs = len(out_avals)
    in_names_full = list(in_names) + out_names
    if partition_name is not None:
        in_names_full.append(partition_name)
    donate = tuple(range(n_params, n_params + n_outs))

    def _body(*args):
        operands = list(args)
        if partition_name is not None:
            operands.append(partition_id_tensor())
        outs = _bass_exec_p.bind(
            *operands,
            out_avals=tuple(out_avals),
            in_names=tuple(in_names_full),
            out_names=tuple(out_names),
            lowering_input_output_aliases=(),
            sim_require_finite=True,
            sim_require_nnan=True,
            nc=nc,
        )
        return tuple(outs)

    devices = jax.devices()[:N_CORES]
    mesh = Mesh(np.asarray(devices), ("core",))
    in_specs = (PartitionSpec("core"),) * (n_params + n_outs)
    out_specs = (PartitionSpec("core"),) * n_outs
    fn = jax.jit(
        shard_map(_body, mesh=mesh, in_specs=in_specs, out_specs=out_specs,
                  check_rep=False),
        donate_argnums=donate,
        keep_unused=True,
    )
    _CACHE.update(nc=nc, fn=fn, in_names=in_names, prev_outs=zero_outs,
                  n_outs=n_outs)


def _host_inputs(x, w_q, w_k, w_v, rel_d, rel_h, rel_w):
    """Per-core inputs, already concatenated along axis 0 (shard layout)."""
    rd = np.asarray(rel_d, np.float32).reshape(21, 3)
    rh = np.asarray(rel_h, np.float32).reshape(21, 3)
    rw = np.asarray(rel_w, np.float32).reshape(22, 3)

    xpad = np.zeros((64, 16, HP, HP), np.float16)
    xpad[:, :, 1:65, 1:65] = x[0]
    # core i's own planes, partition = slot*64 + channel
    bnd_g = np.ascontiguousarray(
        xpad.reshape(64, 8, 2, HW).transpose(1, 2, 0, 3)
    ).reshape(N_CORES * 128, HW)

    # one-hot halo selectors over the 16 gathered planes (gathered plane j =
    # padded depth plane j+1); left halo of core i = plane 2i -> j = 2i-1,
    # right halo = plane 2i+3 -> j = 2i+2; edge cores get all-zero rows.
    hs_g = np.zeros((N_CORES, 128, 16), np.float32)
    for i in range(N_CORES):
        if i > 0:
            hs_g[i, 0:64, 2 * i - 1] = 1.0
        if i < N_CORES - 1:
            hs_g[i, 64:128, 2 * i + 2] = 1.0

    kv27 = np.arange(27)
    kd_i, kh_i, kw_i = kv27 // 9, (kv27 // 3) % 3, kv27 % 3
    B64 = np.empty((64, 27), np.float32)
    B64[:21] = rd[:, kd_i]
    B64[21:42] = rh[:, kh_i]
    B64[42:] = rw[:, kw_i]
    B = np.concatenate([B64, B64], 0)

    import ml_dtypes
    idt = np.eye(128, dtype=np.float32)
    idt = idt.astype(ml_dtypes.bfloat16 if E_BF16 else np.float32)

    def dup(w):
        w2 = np.concatenate([w.T, w.T], 1).astype(np.float16)
        return np.tile(w2, (N_CORES, 1))

    return {
        "bnd": bnd_g,
        "hsel": hs_g.reshape(N_CORES * 128, 16),
        "wk2": dup(np.asarray(w_k)),
        "wv2": dup(np.asarray(w_v)),
        "wq2": dup(np.asarray(w_q)),
        "bias": np.tile(B, (N_CORES, 1)),
        "ident": np.tile(idt, (N_CORES, 1)),
    }


def kernel(x, w_q, w_k, w_v, rel_d, rel_h, rel_w):
    import jax

    if "fn" not in _CACHE:
        _compile()

    gmaps = _host_inputs(np.asarray(x, np.float32), w_q, w_k, w_v,
                         rel_d, rel_h, rel_w)
    args = [gmaps[nm] for nm in _CACHE["in_names"]]
    out_arrs = _CACHE["fn"](*args, *_CACHE["prev_outs"])
    # recycle the device-resident output buffers as next call's donation args
    # (their contents are irrelevant: the NEFF writes every output element)
    _CACHE["prev_outs"] = list(out_arrs)

    o = np.asarray(out_arrs[0])          # [8*128, 64, 64] fp16
    og = o.reshape(N_CORES, 2, 64, 64, 64)
    out = og.transpose(2, 0, 1, 3, 4).reshape(1, 64, 16, 64, 64)
    return out.astype(np.float32)
